# revision 13
# baseline (speedup 1.0000x reference)
"""AttentionBlock (GroupNorm + 8-head self-attention + proj + residual) on 8 trn2 cores.

Sharding: data-parallel over batch B=8 -> one batch per NeuronCore; no collectives.

Schedule (per core): the ScalarE exp stream and the PE matmul stream are the
two walls; the kernel keeps both streaming:
  - x DMA chunks land first; bn_stats per chunk as they arrive; GroupNorm
    istd = rsqrt(var+eps) via DVE bit-trick + 2 Newton steps (ScalarE only
    ever runs Exp/Identity -> exactly one ACT_TABLE_LOAD, preloaded at t~0
    by a dummy exp). PE warmup matmuls keep HAM at full clock through the
    DMA/GN phase (written to unused halves of the GN PSUM tiles).
  - Heads in pairs (head 2m: PE rows 0-63, head 2m+1: rows 64-127). Within
    a pair, logits LEAD the AV matmuls by one j-tile in the PE FIFO so the
    exp stream is never gated behind AV.
  - AV: vT head slot = [ones x64 | v x64] -> PSUM rows 0-63 hold the softmax
    denominator, 64-127 the unnormalized rows. Drain = reciprocal_approx_fast
    + partition-shift copy + one multiply (no broadcast matmul).
  - Some exp tiles per pair run on the DVE via the Schraudolph bit trick
    (uint16 bf16-bits, bitcast to bf16; ~4% max rel err, fine here).
  - q/k for later pairs, vT blocks, and the residual pre-bias are spread
    under the pair-0/1 windows; proj + residual form the tail.
"""

import math
import os
import sys

import numpy as np

for _p in (
    "/opt/trn_rl_repo",
    "/root/.axon_site",
    "/root/.axon_site/_ro/trn_rl_repo",
    "/root/.axon_site/_ro/pypackages",
):
    if os.path.isdir(_p) and _p not in sys.path:
        sys.path.append(_p)

import ml_dtypes  # noqa: E402

import concourse.bass as bass  # noqa: E402
import concourse.mybir as mybir  # noqa: E402
import concourse.tile as tile  # noqa: E402
from concourse import bacc  # noqa: E402

B, C, HH, WW = 8, 512, 32, 32
L = HH * WW  # 1024
NH, CH = 8, 64  # heads, channels per head
G, GS = 32, 16  # groups, channels per group
EPS = 1e-5
P = 128
NT = C // P  # 4 channel tiles
ST = L // P  # 8 s tiles
F32 = mybir.dt.float32
BF16 = mybir.dt.bfloat16
I32 = mybir.dt.int32
U16 = mybir.dt.uint16
FP8 = mybir.dt.float8e4
I8 = mybir.dt.int8
N_CORES = 8

LOG2E = 1.4426950408889634
EXP_A = 8.0 * LOG2E
EXP_B = 8.0 * (7.0 - 0.0579)
RSQRT_MAGIC = 0x5F3759DF

# per-head sets of j whose exp runs on the DVE instead of ScalarE
DVE_J = {
    0: set(), 1: {5}, 2: {2, 5}, 3: {5},
    4: {2, 5}, 5: {5}, 6: {2, 5}, 7: {2, 5, 7},
}


def _emit(tc: tile.TileContext, io: dict):
    nc = tc.nc
    x_d = io["x"].rearrange("(t p) l -> p t l", p=P)
    xb_d = io["x_bf16"].rearrange("(t p) l -> p t l", p=P)
    wqkvT_d = io["wqkvT"].rearrange("(t p) o -> p t o", p=P)
    wprojT_d = io["wprojT"].rearrange("(t p) o -> p t o", p=P)
    # packed small params: [gnw|gnb|bq|bk|bproj (NT each) | bv (4) | indf (128)]
    pk_d = io["packed"]
    out_d = io["out"].rearrange("(t p) l -> p t l", p=P)

    from contextlib import ExitStack

    with ExitStack() as stack:
        persist = stack.enter_context(tc.tile_pool(name="persist", bufs=1))
        work = stack.enter_context(tc.tile_pool(name="work", bufs=2))
        ew_pool = stack.enter_context(tc.tile_pool(name="ew_pool", bufs=8))
        scr_pool = stack.enter_context(tc.tile_pool(name="scr_pool", bufs=2))
        out_pool = stack.enter_context(tc.tile_pool(name="out_pool", bufs=2))
        ps_rot = stack.enter_context(tc.tile_pool(name="ps_rot", bufs=2, space="PSUM"))
        ps_pv = stack.enter_context(tc.tile_pool(name="ps_pv", bufs=2, space="PSUM"))

        # ---- persistent tiles ----
        xt = persist.tile([P, NT, L], F32, name="xt")
        xb = persist.tile([P, NT, L], BF16, name="xb")
        wqkvT = persist.tile([P, NT, 3 * C], FP8, name="wqkvT")
        wprojT = persist.tile([P, NT, C], FP8, name="wprojT")
        pk = persist.tile([P, 5 * NT + G * NT + C], F32, name="pk")
        indb = persist.tile([G, NT, P], F32, name="indb")
        hn = persist.tile([P, NT, L], FP8, name="hn")
        qq = persist.tile([P, NT, L], BF16, name="qq")
        kk = persist.tile([P, NT, L], BF16, name="kk")
        vT = persist.tile([P, ST, NH * 128], FP8, name="vT")
        a_all = persist.tile([P, NT, L], FP8, name="a_all")
        stats2 = persist.tile([G, 2], F32, name="stats2")
        dum = persist.tile([1, 1], F32, name="dum")

        gnw = pk[:, 0:NT].rearrange("p (t one) -> p t one", one=1)
        gnb = pk[:, NT : 2 * NT].rearrange("p (t one) -> p t one", one=1)
        bq = pk[:, 2 * NT : 3 * NT].rearrange("p (t one) -> p t one", one=1)
        bk = pk[:, 3 * NT : 4 * NT].rearrange("p (t one) -> p t one", one=1)
        bproj = pk[:, 4 * NT : 5 * NT].rearrange("p (t one) -> p t one", one=1)
        indf = pk[:, 5 * NT : 5 * NT + G * NT].rearrange("p (t g) -> p t g", g=G)
        bv3 = pk[:, 5 * NT + G * NT :].rearrange("p (h c) -> p h c", c=CH)

        vT_h = vT.rearrange("p s (h x) -> p s h x", x=128)

        # Dummy exp first: hoists the single ACT_TABLE_LOAD to t~0.
        nc.vector.memset(dum[:], 0.0)
        nc.scalar.activation(
            out=dum[:], in_=dum[:], func=mybir.ActivationFunctionType.Exp
        )
        nc.gpsimd.memset(vT_h[:, :, :, 0:64], 1.0)

        # ---- loads: x chunks first (with bn_stats), then weights ----
        psg_t = ps_rot.tile([P, L], F32, name="psg_t", tag="rot")
        psg = psg_t[0:G, 0:2]

        def emit_warm(t, sub):
            # warmup matmul on the freshly-landed x chunk: keeps HAM at full
            # clock through the DMA/GN phase, paced by chunk arrival
            nc.tensor.matmul(
                psg_t[:, 512:1024],
                lhsT=xb[:, t, 0:128],
                rhs=xb[:, t, sub * 512 : (sub + 1) * 512],
                start=True,
                stop=True,
            )

        st6s = []
        for t in range(NT):
            st6 = work.tile([P, 2, 6], F32, name="st6", tag="st6", bufs=NT)
            for sub in range(2):
                nc.sync.dma_start(
                    out=xb[:, t, sub * 512 : (sub + 1) * 512],
                    in_=xb_d[:, t, sub * 512 : (sub + 1) * 512],
                )
                nc.vector.bn_stats(
                    out=st6[:, sub, :], in_=xb[:, t, sub * 512 : (sub + 1) * 512]
                )
                emit_warm(t, sub)
            st6s.append(st6)
        for _ in range(6):
            emit_warm(NT - 1, 1)
        nc.sync.dma_start(out=pk[:], in_=pk_d)
        nc.sync.dma_start(out=indb[:], in_=io["ind_bwd"].rearrange("g (t p) -> g t p", p=P))
        nc.sync.dma_start(out=wqkvT[:, :, 0:512], in_=wqkvT_d[:, :, 0:512])
        nc.sync.dma_start(out=wqkvT[:, :, 512:1024], in_=wqkvT_d[:, :, 512:1024])
        nc.sync.dma_start(out=wqkvT[:, :, 1024:1536], in_=wqkvT_d[:, :, 1024:1536])
        nc.sync.dma_start(out=wprojT[:], in_=wprojT_d)
        for t in range(NT):  # f32 x for the residual tail; off critical path
            nc.sync.dma_start(out=xt[:, t, :], in_=x_d[:, t, :])

        # ---- GroupNorm stats ----
        mm2s = []
        for t in range(NT):
            mm2 = work.tile([P, 2], F32, name="mm2", tag="mm2", bufs=NT)
            nc.vector.bn_aggr(out=mm2[:], in_=st6s[t][:])  # [mean_c, var_c]
            sq = work.tile([P, 1], F32, name="sq", tag="sq")
            nc.vector.tensor_mul(out=sq[:], in0=mm2[:, 0:1], in1=mm2[:, 0:1])
            nc.vector.tensor_add(out=mm2[:, 1:2], in0=mm2[:, 1:2], in1=sq[:])
            mm2s.append(mm2)
        for t in range(NT):
            nc.tensor.matmul(
                psg[:],
                lhsT=indf[:, t, :],
                rhs=mm2s[t][:],
                start=(t == 0),
                stop=(t == NT - 1),
            )
        # istd = rsqrt(var+eps): bit-trick seed + 2 Newton iterations (DVE only)
        nc.vector.tensor_copy(out=stats2[:, 0:1], in_=psg[:, 0:1])
        sqg = work.tile([G, 1], F32, name="sqg", tag="sqg")
        nc.vector.tensor_mul(out=sqg[:], in0=stats2[:, 0:1], in1=stats2[:, 0:1])
        varg = work.tile([G, 1], F32, name="varg", tag="varg")
        nc.vector.tensor_sub(out=varg[:], in0=psg[:, 1:2], in1=sqg[:])
        nc.vector.tensor_scalar_add(out=varg[:], in0=varg[:], scalar1=EPS)
        ti = work.tile([G, 1], I32, name="ti", tag="ti")
        nc.vector.tensor_scalar(
            out=ti[:], in0=varg[:].bitcast(I32), scalar1=1, scalar2=None,
            op0=mybir.AluOpType.arith_shift_right,
        )
        nc.vector.tensor_scalar(
            out=ti[:], in0=ti[:], scalar1=-1, scalar2=float(RSQRT_MAGIC),
            op0=mybir.AluOpType.mult, op1=mybir.AluOpType.add,
        )
        y0 = ti[:].bitcast(F32)
        yt = work.tile([G, 1], F32, name="yt", tag="yt")
        for it in range(2):
            dst = stats2[:, 1:2] if it == 1 else None
            nc.vector.tensor_mul(out=yt[:], in0=varg[:], in1=y0)
            nc.vector.tensor_mul(out=yt[:], in0=yt[:], in1=y0)
            nc.vector.tensor_scalar(
                out=yt[:], in0=yt[:], scalar1=-0.5, scalar2=1.5,
                op0=mybir.AluOpType.mult, op1=mybir.AluOpType.add,
            )
            if it == 0:
                nc.vector.tensor_mul(out=ti[:].bitcast(F32), in0=y0, in1=yt[:])
            else:
                nc.vector.tensor_mul(out=stats2[:, 1:2], in0=y0, in1=yt[:])

        # ---- GN apply: hn = x * sc + tc (2 tiles ScalarE, 2 tiles DVE) ----
        for t in range(NT):
            psb_t = ps_rot.tile([P, L], F32, name="psb_t", tag="rot")
            psb = psb_t[0:P, 0:2]
            nc.tensor.matmul(
                psb[:], lhsT=indb[:, t, :], rhs=stats2[:], start=True, stop=True
            )
            sc = work.tile([P, 1], F32, name="sc", tag="sc", bufs=4)
            nc.vector.tensor_mul(out=sc[:], in0=psb[:, 1:2], in1=gnw[:, t, :])
            tc_ = work.tile([P, 1], F32, name="tc_", tag="tc_", bufs=4)
            nc.vector.tensor_mul(out=tc_[:], in0=psb[:, 0:1], in1=sc[:])
            nc.vector.tensor_sub(out=tc_[:], in0=gnb[:, t, :], in1=tc_[:])
            if t < 2:
                nc.scalar.activation(
                    out=hn[:, t, :],
                    in_=xb[:, t, :],
                    func=mybir.ActivationFunctionType.Identity,
                    bias=tc_[:],
                    scale=sc[:],
                )
            else:
                nc.vector.tensor_scalar(
                    out=hn[:, t, :],
                    in0=xb[:, t, :],
                    scalar1=sc[:],
                    scalar2=tc_[:],
                    op0=mybir.AluOpType.mult,
                    op1=mybir.AluOpType.add,
                )

        # ---- helpers ----
        def emit_qk_mm(m, which):
            dest, bias, ofs = (
                (qq, bq, 0) if which == "q" else (kk, bk, C)
            )
            ps = ps_rot.tile([P, L], F32, name=f"ps{which}{m}", tag="rot")
            for kp in range(2):
                for n in range(2):
                    nc.tensor.matmul(
                        ps[:, n * 512 : (n + 1) * 512],
                        lhsT=wqkvT[:, 2 * kp : 2 * kp + 2, ofs + m * P : ofs + (m + 1) * P],
                        rhs=hn[:, 2 * kp : 2 * kp + 2, n * 512 : (n + 1) * 512],
                        start=(kp == 0),
                        stop=(kp == 1),
                        perf_mode=mybir.MatmulPerfMode.DoubleRow,
                    )
            nc.vector.tensor_scalar(
                out=dest[:, m, :],
                in0=ps[:],
                scalar1=bias[:, m, :],
                scalar2=None,
                op0=mybir.AluOpType.add,
            )

        def emit_vt(s):
            psv = ps_rot.tile([P, L], F32, name=f"psvT{s}", tag="rot")
            for kp in range(2):
                nc.tensor.matmul(
                    psv[:, 0:512],
                    lhsT=hn[:, 2 * kp : 2 * kp + 2, s * P : (s + 1) * P],
                    rhs=wqkvT[:, 2 * kp : 2 * kp + 2, 2 * C : 3 * C],
                    start=(kp == 0),
                    stop=(kp == 1),
                    perf_mode=mybir.MatmulPerfMode.DoubleRow,
                )
            nc.vector.tensor_tensor(
                out=vT_h[:, s, :, 64:128],
                in0=psv[:, 0:512].rearrange("p (h c) -> p h c", c=CH),
                in1=bv3,
                op=mybir.AluOpType.add,
            )

        def emit_drain(h, pv):
            """a = a' / D. pv rows 0-63 all hold D; rows 64-127 hold a'."""
            pr, part = h // 2, (h % 2) * 64
            scr = scr_pool.tile([P, L], F32, name=f"scr{h}", tag="scr")
            nc.vector.reciprocal_approx_fast(out=scr[0:64, :], in_=pv[0:64, :])
            nc.vector.tensor_copy(out=scr[64:128, :], in_=scr[0:64, :])
            nc.vector.tensor_tensor(
                out=a_all[part : part + 64, pr, :],
                in0=pv[64:128, :],
                in1=scr[64:128, :],
                op=mybir.AluOpType.mult,
            )

        def emit_prebias(t):
            nc.vector.tensor_scalar_add(
                out=xt[:, t, :], in0=xt[:, t, :], scalar1=bproj[:, t, :]
            )

        # ---- attention: one head at a time; exp stream paces the kernel ----
        emit_qk_mm(0, "q")
        emit_qk_mm(0, "k")
        emit_vt(0)
        emit_vt(1)

        prev = None  # (head, pv)
        for h in range(NH):
            m, part = h // 2, (h % 2) * 64
            pv = ps_pv.tile([P, L], F32, name=f"pv{h}", tag="pv")
            ew_pair = {}
            for j in range(ST):
                jp, sub = j // 2, j % 2
                psL = ps_rot.tile([P, L], F32, name=f"pg{h}{j}", tag="rot")
                for n in range(2):
                    nc.tensor.matmul(
                        psL[:, n * 512 : (n + 1) * 512],
                        lhsT=kk[part : part + 64, m, j * P : (j + 1) * P],
                        rhs=qq[part : part + 64, m, n * 512 : (n + 1) * 512],
                        start=True,
                        stop=True,
                        tile_position=(part, 0),
                    )
                if sub == 0:
                    ew_pair[jp] = ew_pool.tile([P, 2, L], FP8, name=f"ew{h}{jp}", tag="ew")
                ewt = ew_pair[jp]
                if j in DVE_J[h]:
                    nc.vector.tensor_scalar(
                        out=ewt[:, sub, :].bitcast(I8),
                        in0=psL[:],
                        scalar1=EXP_A,
                        scalar2=EXP_B,
                        op0=mybir.AluOpType.mult,
                        op1=mybir.AluOpType.add,
                    )
                else:
                    nc.scalar.activation(
                        out=ewt[:, sub, :],
                        in_=psL[:],
                        func=mybir.ActivationFunctionType.Exp,
                    )

                def emit_av(jjp):
                    for n in range(2):
                        nc.tensor.matmul(
                            pv[:, n * 512 : (n + 1) * 512],
                            lhsT=vT[:, 2 * jjp : 2 * jjp + 2, h * 128 : (h + 1) * 128],
                            rhs=ew_pair[jjp][:, :, n * 512 : (n + 1) * 512],
                            start=(jjp == 0),
                            stop=(jjp == ST // 2 - 1),
                            perf_mode=mybir.MatmulPerfMode.DoubleRow,
                        )

                if j >= 2 and j % 2 == 0:
                    emit_av(j // 2 - 1)
                # deferred drain of the previous head, early in this head's DVE queue
                if prev is not None and j == 0:
                    emit_drain(*prev)
                # spread prep work under the early heads' windows
                if h == 0 and j < 6:
                    emit_vt(j + 2)
                if h == 1 and j == 1:
                    emit_qk_mm(1, "q")
                if h == 1 and j == 4:
                    emit_qk_mm(1, "k")
                if h == 2 and j == 1:
                    emit_prebias(0)
                if h == 2 and j == 4:
                    emit_prebias(1)
                if h == 3 and j == 1:
                    emit_qk_mm(2, "q")
                if h == 3 and j == 4:
                    emit_qk_mm(2, "k")
                if h == 4 and j == 1:
                    emit_prebias(2)
                if h == 4 and j == 4:
                    emit_prebias(3)
                if h == 5 and j == 1:
                    emit_qk_mm(3, "q")
                if h == 5 and j == 4:
                    emit_qk_mm(3, "k")
            emit_av(ST // 2 - 1)
            prev = (h, pv)
        emit_drain(*prev)

        # ---- proj + residual (xt pre-biased with proj_b) ----
        for mo in range(NT):
            ps = ps_rot.tile([P, L], F32, name=f"pspj{mo}", tag="rot")
            for n in range(2):
                for kp in range(2):
                    nc.tensor.matmul(
                        ps[:, n * 512 : (n + 1) * 512],
                        lhsT=wprojT[:, 2 * kp : 2 * kp + 2, mo * P : (mo + 1) * P],
                        rhs=a_all[:, 2 * kp : 2 * kp + 2, n * 512 : (n + 1) * 512],
                        start=(kp == 0),
                        stop=(kp == 1),
                        perf_mode=mybir.MatmulPerfMode.DoubleRow,
                    )
            ot = out_pool.tile([P, L], F32, name=f"ot{mo}", tag="ot")
            for n in range(2):
                sl = slice(n * 512, (n + 1) * 512)
                nc.vector.tensor_tensor(
                    out=ot[:, sl], in0=ps[:, sl], in1=xt[:, mo, sl],
                    op=mybir.AluOpType.add,
                )
                nc.sync.dma_start(out=out_d[:, mo, sl], in_=ot[:, sl])

        if os.environ.get("AB_DEBUG"):
            for nm, t_ in (("d_hn", hn), ("d_qq", qq), ("d_kk", kk),
                           ("d_aall", a_all)):
                nc.sync.dma_start(
                    out=io[nm].rearrange("(t p) l -> p t l", p=P), in_=t_[:]
                )
            nc.sync.dma_start(out=io["d_stats"], in_=stats2[:])
            nc.sync.dma_start(out=io["d_vt"].rearrange("p (s x) -> p s x", s=ST), in_=vT[:])


def build_nc() -> bass.Bass:
    nc = bacc.Bacc("TRN2", target_bir_lowering=False, debug=False)
    io = {}
    specs = [
        ("x", [C, L], F32),
        ("x_bf16", [C, L], BF16),
        ("wqkvT", [C, 3 * C], FP8),
        ("wprojT", [C, C], FP8),
        ("packed", [P, 5 * NT + G * NT + C], F32),
        ("ind_bwd", [G, C], F32),
    ]
    for name, shape, dt in specs:
        io[name] = nc.declare_dram_parameter(name, shape, dt, isOutput=False).ap()
    io["out"] = nc.declare_dram_parameter("out", [C, L], F32, isOutput=True).ap()
    if os.environ.get("AB_DEBUG"):
        for nm, shape, dt in (
            ("d_hn", [C, L], BF16), ("d_qq", [C, L], BF16), ("d_kk", [C, L], BF16),
            ("d_aall", [C, L], BF16), ("d_stats", [G, 2], F32),
            ("d_vt", [P, ST * NH * 128], BF16),
        ):
            io[nm] = nc.declare_dram_parameter(nm, shape, dt, isOutput=True).ap()
    with tile.TileContext(nc) as tc:
        _emit(tc, io)
    nc.compile()
    return nc


def host_prepare(inputs: dict) -> list[dict]:
    """Full inputs -> per-core in_maps (shard batch, reorder/transpose weights)."""
    x = np.ascontiguousarray(np.asarray(inputs["x"], dtype=np.float32))
    gn_w = np.asarray(inputs["gn_w"], dtype=np.float32)
    gn_b = np.asarray(inputs["gn_b"], dtype=np.float32)
    qkv_w = np.asarray(inputs["qkv_w"], dtype=np.float32)
    qkv_b = np.asarray(inputs["qkv_b"], dtype=np.float32)
    proj_w = np.asarray(inputs["proj_w"], dtype=np.float32)
    proj_b = np.asarray(inputs["proj_b"], dtype=np.float32)

    s2 = 1.0 / math.sqrt(CH)  # folded double-softmax scale
    w3 = qkv_w.reshape(NH, 3, CH, C)
    b3 = qkv_b.reshape(NH, 3, CH)
    wq = w3[:, 0].reshape(C, C) * s2
    wk = w3[:, 1].reshape(C, C)
    wv = w3[:, 2].reshape(C, C)
    wqkvT = np.concatenate([wq, wk, wv], 0).T.astype(ml_dtypes.float8_e4m3)
    wqkvT = np.ascontiguousarray(wqkvT)
    wprojT = np.ascontiguousarray(proj_w.T.astype(ml_dtypes.float8_e4m3))

    # packed [P, 5*NT + 4 + G*NT] f32:
    #   gnw/gnb/bq/bk/bproj as (NT, P) columns; bv as (P -> head, ch) 4 cols;
    #   indf as (NT, G) blocks
    def col(v):  # (C,) -> (P, NT) channel tiling
        return np.ascontiguousarray(v.reshape(NT, P).T)

    bq_v = (b3[:, 0].reshape(C) * s2)
    bk_v = b3[:, 1].reshape(C)
    bv_v = b3[:, 2].reshape(C)
    bv_rep = np.broadcast_to(bv_v.reshape(1, C), (P, C)).astype(np.float32)
    cc = np.arange(C)
    gg = np.arange(G)
    ind_fwd = ((cc[:, None] // GS) == gg[None, :]).astype(np.float32) / GS
    indf_p = ind_fwd.reshape(NT, P, G).transpose(1, 0, 2).reshape(P, NT * G)

    packed = np.concatenate(
        [
            col(gn_w.reshape(C)),
            col(gn_b.reshape(C)),
            col(bq_v),
            col(bk_v),
            col(proj_b.reshape(C)),
            indf_p,
            bv_rep,
        ],
        axis=1,
    ).astype(np.float32)
    packed = np.ascontiguousarray(packed)

    ind_bwd = np.ascontiguousarray(ind_fwd.T) * GS
    shared = dict(wqkvT=wqkvT, wprojT=wprojT, packed=packed, ind_bwd=ind_bwd)
    return [
        dict(
            shared,
            x=np.ascontiguousarray(x[b].reshape(C, L)),
            x_bf16=np.ascontiguousarray(x[b].reshape(C, L).astype(ml_dtypes.bfloat16)),
        )
        for b in range(B)
    ]


_NC_CACHE = None


def _get_nc():
    global _NC_CACHE
    if _NC_CACHE is None:
        _NC_CACHE = build_nc()
    return _NC_CACHE


def kernel(**inputs) -> np.ndarray:
    from concourse.bass_utils import run_bass_kernel_spmd

    in_maps = host_prepare(inputs)
    res = run_bass_kernel_spmd(_get_nc(), in_maps, list(range(N_CORES)))
    outs = [np.asarray(res.results[i]["out"], dtype=np.float32) for i in range(N_CORES)]
    return np.stack(outs, 0).reshape(B, C, HH, WW)


if __name__ == "__main__":
    d = np.load("/tmp/inputs.npz")
    out = kernel(**{k: d[k] for k in d.files})
    ref = np.load("/tmp/ref.npy")
    rel = np.linalg.norm(out - ref) / np.linalg.norm(ref)
    print("Relative error:", rel)
